# revision 7
# baseline (speedup 1.0000x reference)
"""Trainium2 Bass kernel for CURLoRA forward: out = x @ (C @ U @ R).T

Fused low-rank chain per core (never materializes the [8192, 8192] W):
  t1.T = sum_k R_k.T.T @ x_k.T     (64 K-tiles of 128, f32r, PSUM-accumulated)
  t2.T = [U.T|U.T].T @ t1.T        (f32r matmul, M=128 duplicates t2.T
                                    into both partition halves)
  out  = t2.T.T @ C.T              (fp32, 2x2 tile_position-packed quadrant
                                    matmuls into four [128,512] PSUM banks)

Sharding (8 cores, no collectives): the 128 rows of x are split 4 ways and
the 8192 output columns 2 ways. Per core DMA: 1MB x-shard + 2MB R
(replicated; irreducible without cross-core comms) + 1MB C.T shard + 0.5MB
out = 4.5MB vs 6.75MB for the "shard C rows only" layout. All transposes
are host-side layout prep during sharding; every FLOP runs on-device.

Schedule: the contraction is split at k=32 and stages 2+3 run TWICE with
PSUM accumulation, so the first half's stage-2/3 work happens while the
second half of x/R still streams in; after the last input byte only the
half-2 tail (stage1 tail + small stage2 + half of stage3 + copy + out DMA)
remains. DMA queues are balanced across the three DGE paths (gpsimd: uq+x,
sync: x tail + C, scalar: R) so the aggregate stream runs at the per-core
HBM roofline with C in place before mid-stream stage 3a. Output is written
from two HWDGE rings (sync+scalar) as soon as each 256-col block's PSUM
copy lands. One semaphore per DMA stream: completions on one queue from
one engine are FIFO, so inc-by-16 thresholds are sound.
"""

import numpy as np

B, S, M, N, RANK = 2, 64, 8192, 8192, 64
NCORES = 8
SA, NB = 4, 2              # s-blocks x n-blocks = 8 cores
SSH = (B * S) // SA        # 32 s-rows per core
NSH = N // NB              # 4096 out cols per core
KCH = M // 128             # 64 contraction chunks of 128

# k-chunks per x/R piece; stage 2a/3a runs after piece 1 (k=32)
PIECES = (8, 24, 24, 8)

_NC_CACHE = {}


def _build_nc():
    if "nc" in _NC_CACHE:
        return _NC_CACHE["nc"]
    from contextlib import ExitStack
    from concourse import mybir
    import concourse.bass as bass

    f32 = mybir.dt.float32
    f32r = mybir.dt.float32r
    nc = bass.Bass()

    xp_d = nc.declare_dram_parameter("xp", [128, KCH * SSH], f32r, isOutput=False)
    rp_d = nc.declare_dram_parameter("rp", [128, KCH * RANK], f32r, isOutput=False)
    uq_d = nc.declare_dram_parameter("uq", [RANK, 128], f32r, isOutput=False)
    ct_d = nc.declare_dram_parameter("ct", [128, NSH // 2], f32, isOutput=False)
    out_d = nc.declare_dram_parameter("out", [128, NSH // 4], f32, isOutput=True)

    ctx = ExitStack()
    with ctx:
        xts = [
            ctx.enter_context(nc.sbuf_tensor(f"xt{i}", [128, kw * SSH], f32r))
            for i, kw in enumerate(PIECES)
        ]
        rts = [
            ctx.enter_context(nc.sbuf_tensor(f"rt{i}", [128, kw * RANK], f32r))
            for i, kw in enumerate(PIECES)
        ]
        uqt = ctx.enter_context(nc.sbuf_tensor("uqt", [RANK, 128], f32r))
        cts = [
            ctx.enter_context(nc.sbuf_tensor(f"ct{i}", [128, 1024], f32))
            for i in range(2)
        ]
        t1s = [
            ctx.enter_context(nc.sbuf_tensor(f"t1s{h}", [RANK, SSH], f32r))
            for h in range(2)
        ]
        t2s = [
            ctx.enter_context(nc.sbuf_tensor(f"t2s{h}", [128, SSH], f32))
            for h in range(2)
        ]
        osbs = [
            ctx.enter_context(nc.sbuf_tensor(f"osb{i}", [128, 256], f32))
            for i in range(4)
        ]
        ps1 = ctx.enter_context(nc.psum_tensor("ps1", [128, 512], f32))
        ps2 = ctx.enter_context(nc.psum_tensor("ps2", [128, 512], f32))
        psos = [
            ctx.enter_context(nc.psum_tensor(f"pso{i}", [128, 512], f32))
            for i in range(4)
        ]

        # one semaphore per input DMA: an intermediate count on a sem shared
        # by several same-queue DMAs is unsound (each of the 16 SDMA engines
        # incs once per DMA and engines skew, so count>=16 can mix incs from
        # different DMAs). A shared sem at the FULL total is sound (so1/so2).
        sxs = [ctx.enter_context(nc.semaphore(f"sx{i}")) for i in range(4)]
        srs = [ctx.enter_context(nc.semaphore(f"sr{i}")) for i in range(4)]
        su = ctx.enter_context(nc.semaphore("su"))    # uq
        sc = ctx.enter_context(nc.semaphore("sc"))    # ct0+ct1 (wait at 32)
        sm = ctx.enter_context(nc.semaphore("sm"))    # tensor -> vector
        sv = ctx.enter_context(nc.semaphore("sv"))    # vector -> tensor/out
        so1 = ctx.enter_context(nc.semaphore("so1"))  # out DMAs on sync
        so2 = ctx.enter_context(nc.semaphore("so2"))  # out DMAs on scalar

        block = ctx.enter_context(nc.Block())

        # x/R dram column offsets per piece
        xoff = [0]
        for kw in PIECES:
            xoff.append(xoff[-1] + kw)

        @block.gpsimd
        def _(g):
            g.dma_start(uqt[:], uq_d[:]).then_inc(su, 16)
            for p in range(3):  # x pieces 0..2
                g.dma_start(
                    xts[p][:], xp_d[:, xoff[p] * SSH:xoff[p + 1] * SSH]
                ).then_inc(sxs[p], 16)

        @block.sync
        def _(sync):
            sync.dma_start(
                xts[3][:], xp_d[:, xoff[3] * SSH:xoff[4] * SSH]
            ).then_inc(sxs[3], 16)
            sync.dma_start(cts[0][:], ct_d[:, 0:1024]).then_inc(sc, 16)
            sync.dma_start(cts[1][:], ct_d[:, 1024:2048]).then_inc(sc, 16)
            for cb in range(2):
                sync.wait_ge(sv, 5 + cb)
                sync.dma_start(
                    out_d[:, cb * 256:(cb + 1) * 256], osbs[cb][:]
                ).then_inc(so1, 16)
            sync.wait_ge(so1, 32)

        @block.scalar
        def _(scalar):
            for p in range(4):
                scalar.dma_start(
                    rts[p][:], rp_d[:, xoff[p] * RANK:xoff[p + 1] * RANK]
                ).then_inc(srs[p], 16)
            for cb in range(2, 4):
                scalar.wait_ge(sv, 5 + cb)
                scalar.dma_start(
                    out_d[:, cb * 256:(cb + 1) * 256], osbs[cb][:]
                ).then_inc(so2, 16)
            scalar.wait_ge(so2, 32)

        def stage1(t, pieces, k0):
            """PSUM-accumulate R_p.T.T @ x_p.T for the given pieces."""
            k = k0
            last_mm = None
            for p in pieces:
                t.wait_ge(sxs[p], 16)            # x piece p
                t.wait_ge(srs[p], 16)            # R piece p
                for kl in range(PIECES[p]):
                    last_mm = nc.tensor.matmul(
                        ps1[0:RANK, 0:SSH],
                        rts[p][:, kl * RANK:(kl + 1) * RANK],
                        xts[p][:, kl * SSH:(kl + 1) * SSH],
                        start=(k == k0), stop=(k == k0 + 31),
                    )
                    k += 1
            return last_mm

        def stage3(t, h, sm_base):
            """Quadrant-packed out += t2s[h].T @ C.T; h=0 opens the
            accumulation groups, h=1 closes them and signals vector."""
            for cb in range(4):
                rh, hb = cb // 2, cb % 2
                last_mm = None
                for p in range(2):
                    for w in range(2):
                        q = p * 2 + w
                        c0 = w * 512 + hb * 256
                        last_mm = nc.tensor.matmul(
                            psos[cb][q * SSH:(q + 1) * SSH, 0:256],
                            t2s[h][rh * 64:(rh + 1) * 64, :],
                            cts[p][rh * 64:(rh + 1) * 64, c0:c0 + 256],
                            start=(h == 0), stop=(h == 1),
                            tile_position=(rh * 64, q * SSH),
                        )
                if h == 1:
                    last_mm.then_inc(sm, 1)      # sm=sm_base+cb

        @block.tensor
        def _(t):
            stage1(t, (0, 1), 0).then_inc(sm, 1)            # sm=1: t1a ready
            t.wait_ge(sv, 1)                                # t1a copied
            t.wait_ge(su, 16)                               # uqt loaded
            nc.tensor.matmul(ps2[:, 0:SSH], uqt[:], t1s[0][:],
                             start=True, stop=True).then_inc(sm, 1)  # sm=2
            t.wait_ge(sv, 2)                                # t2a copied
            t.wait_ge(sc, 32)                               # ct0+ct1 loaded
            stage3(t, 0, None)                              # open accum groups
            stage1(t, (2, 3), 32).then_inc(sm, 1)           # sm=3: t1b ready
            t.wait_ge(sv, 3)                                # t1b copied
            nc.tensor.matmul(ps2[:, 0:SSH], uqt[:], t1s[1][:],
                             start=True, stop=True).then_inc(sm, 1)  # sm=4
            t.wait_ge(sv, 4)                                # t2b copied
            stage3(t, 1, 5)                                 # close; sm=5..8

        @block.vector
        def _(v):
            for h in range(2):
                v.wait_ge(sm, 1 + 2 * h)
                nc.vector.tensor_copy(
                    t1s[h][:], ps1[0:RANK, 0:SSH]
                ).then_inc(sv, 1)
                v.wait_ge(sm, 2 + 2 * h)
                nc.vector.tensor_copy(
                    t2s[h][:], ps2[:, 0:SSH]
                ).then_inc(sv, 1)
            for cb in range(4):
                v.wait_ge(sm, 5 + cb)
                nc.vector.tensor_copy(
                    osbs[cb][:], psos[cb][:, 0:256]
                ).then_inc(sv, 1)

    _NC_CACHE["nc"] = nc
    return nc


def _shard_inputs(x, C, U, R):
    xf = np.asarray(x, np.float32).reshape(B * S, M)
    C = np.asarray(C, np.float32)
    U = np.asarray(U, np.float32)
    R = np.asarray(R, np.float32)

    # rp[p, k*64+r] = R[r, 128k+p]
    rp = np.ascontiguousarray(
        R.reshape(RANK, KCH, 128).transpose(2, 1, 0)
    ).reshape(128, KCH * RANK)
    # uq = U.T duplicated along columns: stage 2's lhsT, M=128 so t2.T lands
    # duplicated in both partition halves (stage 3 reads them as row halves)
    uq = np.ascontiguousarray(np.concatenate([U.T, U.T], axis=1))

    in_maps = []
    for c in range(NCORES):
        i, j = divmod(c, NB)
        xs = xf[i * SSH:(i + 1) * SSH, :]
        # xp[p, k*32+s] = xs[s, 128k+p]
        xp = np.ascontiguousarray(
            xs.reshape(SSH, KCH, 128).transpose(2, 1, 0)
        ).reshape(128, KCH * SSH)
        # ct rows 0:64 = C.T cols [0,2048) of this n-shard, rows 64:128 =
        # cols [2048,4096) -- full 128-partition (= full-bandwidth) DMA
        cT = C[j * NSH:(j + 1) * NSH, :].T  # [64, 4096]
        ct = np.ascontiguousarray(
            np.concatenate([cT[:, :2048], cT[:, 2048:]], axis=0)
        )  # [128, 2048]
        in_maps.append({"xp": xp, "rp": rp, "uq": uq, "ct": ct})
    return in_maps


def _unshard_output(core_outs):
    full = np.empty((B * S, N), np.float32)
    for c in range(NCORES):
        i, j = divmod(c, NB)
        q = core_outs[c]  # [128, 1024]: q[32a+s, 512h+nr] = out[s, (4h+a)*512+nr]
        blk = q.reshape(4, SSH, 2, 512).transpose(1, 2, 0, 3).reshape(SSH, NSH)
        full[i * SSH:(i + 1) * SSH, j * NSH:(j + 1) * NSH] = blk
    return full.reshape(B, S, N)


def _ensure_ntff_hook():
    """bass_utils' axon trace path imports antenv.axon_hooks, which this
    container's antenv lacks. Register an equivalent module backed by the
    boot package's ctypes NTFF hook so trace=True (or BASS_TRACE=1) works."""
    import sys
    import types

    try:
        from antenv.axon_hooks import get_axon_ntff_profile_hook  # noqa: F401
        return
    except ImportError:
        pass
    try:
        from trn_agent_boot.trn_boot import _ntff_profile_via_ctypes

        hook = _ntff_profile_via_ctypes("/opt/axon/libaxon_pjrt.so")
    except Exception:
        hook = None
    mod = types.ModuleType("antenv.axon_hooks")
    state = {"hook": hook}
    mod.get_axon_ntff_profile_hook = lambda: state["hook"]
    mod.set_axon_ntff_profile_hook = lambda h: state.update(hook=h)
    sys.modules["antenv.axon_hooks"] = mod


def run(x, C, U, R, trace=False, **spmd_kwargs):
    from concourse.bass_utils import run_bass_kernel_spmd

    _ensure_ntff_hook()
    nc = _build_nc()
    in_maps = _shard_inputs(x, C, U, R)
    res = run_bass_kernel_spmd(
        nc, in_maps, core_ids=list(range(NCORES)), trace=trace, **spmd_kwargs
    )
    out = _unshard_output([r["out"] for r in res.results])
    return out, res


def kernel(x, C, U, R):
    out, _ = run(x, C, U, R, trace=False)
    return out


# revision 8
# speedup vs baseline: 1.1350x; 1.1350x over previous
"""Trainium2 Bass kernel for CURLoRA forward: out = x @ (C @ U @ R).T

Fused low-rank chain per core (never materializes the [8192, 8192] W):
  t1.T = sum_k R_k.T.T @ x_k.T     (64 K-tiles of 128, f32r, PSUM-accumulated)
  t2.T = [U.T|U.T].T @ t1.T        (f32r matmul, M=128 duplicates t2.T
                                    into both partition halves)
  out  = t2.T.T @ C.T              (fp32, 2x2 tile_position-packed quadrant
                                    matmuls into four [128,512] PSUM banks)

Sharding (8 cores, no collectives): the 128 rows of x are split 4 ways and
the 8192 output columns 2 ways. Per core DMA: 1MB x-shard + 2MB R
(replicated; irreducible without cross-core comms) + 1MB C.T shard + 0.5MB
out = 4.5MB vs 6.75MB for the "shard C rows only" layout. All transposes
are host-side layout prep during sharding; every FLOP runs on-device.

Schedule: the contraction is split at k=32 and stages 2+3 run TWICE with
PSUM accumulation, so the first half's stage-2/3 work happens while the
second half of x/R still streams in; after the last input byte only the
half-2 tail (stage1 tail + small stage2 + half of stage3 + copy + out DMA)
remains. DMA queues are balanced across the three DGE paths (gpsimd: uq+x,
sync: x tail + C, scalar: R) so the aggregate stream runs at the per-core
HBM roofline with C in place before mid-stream stage 3a. Output is written
from two HWDGE rings (sync+scalar) as soon as each 256-col block's PSUM
copy lands. One semaphore per DMA stream: completions on one queue from
one engine are FIFO, so inc-by-16 thresholds are sound.
"""

import numpy as np

B, S, M, N, RANK = 2, 64, 8192, 8192, 64
NCORES = 8
SA, NB = 4, 2              # s-blocks x n-blocks = 8 cores
SSH = (B * S) // SA        # 32 s-rows per core
NSH = N // NB              # 4096 out cols per core
KCH = M // 128             # 64 contraction chunks of 128

# k-chunks per x/R piece; stage 2a/3a runs after piece 1 (k=32)
PIECES = (8, 24, 24, 8)

_NC_CACHE = {}


def _build_nc():
    if "nc" in _NC_CACHE:
        return _NC_CACHE["nc"]
    from contextlib import ExitStack
    from concourse import mybir
    import concourse.bass as bass

    f32 = mybir.dt.float32
    f32r = mybir.dt.float32r
    nc = bass.Bass()

    xp_d = nc.declare_dram_parameter("xp", [128, KCH * SSH], f32r, isOutput=False)
    rp_d = nc.declare_dram_parameter("rp", [128, KCH * RANK], f32r, isOutput=False)
    uq_d = nc.declare_dram_parameter("uq", [RANK, 128], f32r, isOutput=False)
    ct_d = nc.declare_dram_parameter("ct", [128, NSH // 2], f32, isOutput=False)
    out_d = nc.declare_dram_parameter("out", [128, NSH // 4], f32, isOutput=True)

    ctx = ExitStack()
    with ctx:
        xts = [
            ctx.enter_context(nc.sbuf_tensor(f"xt{i}", [128, kw * SSH], f32r))
            for i, kw in enumerate(PIECES)
        ]
        rts = [
            ctx.enter_context(nc.sbuf_tensor(f"rt{i}", [128, kw * RANK], f32r))
            for i, kw in enumerate(PIECES)
        ]
        uqt = ctx.enter_context(nc.sbuf_tensor("uqt", [RANK, 128], f32r))
        cts = [
            ctx.enter_context(nc.sbuf_tensor(f"ct{i}", [128, 1024], f32))
            for i in range(2)
        ]
        t1s = [
            ctx.enter_context(nc.sbuf_tensor(f"t1s{h}", [RANK, SSH], f32r))
            for h in range(2)
        ]
        t2s = [
            ctx.enter_context(nc.sbuf_tensor(f"t2s{h}", [128, SSH], f32))
            for h in range(2)
        ]
        osbs = [
            ctx.enter_context(nc.sbuf_tensor(f"osb{i}", [128, 256], f32))
            for i in range(4)
        ]
        ps1 = ctx.enter_context(nc.psum_tensor("ps1", [128, 512], f32))
        ps2 = ctx.enter_context(nc.psum_tensor("ps2", [128, 512], f32))
        psos = [
            ctx.enter_context(nc.psum_tensor(f"pso{i}", [128, 512], f32))
            for i in range(4)
        ]

        # one semaphore per input DMA: an intermediate count on a sem shared
        # by several same-queue DMAs is unsound (each of the 16 SDMA engines
        # incs once per DMA and engines skew, so count>=16 can mix incs from
        # different DMAs). A shared sem at the FULL total is sound (so1/so2).
        sxs = [ctx.enter_context(nc.semaphore(f"sx{i}")) for i in range(4)]
        srs = [ctx.enter_context(nc.semaphore(f"sr{i}")) for i in range(4)]
        su = ctx.enter_context(nc.semaphore("su"))    # uq
        sc = ctx.enter_context(nc.semaphore("sc"))    # ct0+ct1 (wait at 32)
        sm = ctx.enter_context(nc.semaphore("sm"))    # tensor -> vector
        sv = ctx.enter_context(nc.semaphore("sv"))    # vector -> tensor/out
        so1 = ctx.enter_context(nc.semaphore("so1"))  # out DMAs on sync
        so2 = ctx.enter_context(nc.semaphore("so2"))  # out DMAs on scalar

        block = ctx.enter_context(nc.Block())

        # x/R dram column offsets per piece
        xoff = [0]
        for kw in PIECES:
            xoff.append(xoff[-1] + kw)

        @block.gpsimd
        def _(g):
            g.dma_start(uqt[:], uq_d[:]).then_inc(su, 16)
            g.dma_start(cts[0][:], ct_d[:, 0:1024]).then_inc(sc, 16)
            g.dma_start(cts[1][:], ct_d[:, 1024:2048]).then_inc(sc, 16)

        @block.sync
        def _(sync):
            for p in range(4):
                sync.dma_start(
                    xts[p][:], xp_d[:, xoff[p] * SSH:xoff[p + 1] * SSH]
                ).then_inc(sxs[p], 16)
            for cb in range(2):
                sync.wait_ge(sv, 5 + cb)
                sync.dma_start(
                    out_d[:, cb * 256:(cb + 1) * 256], osbs[cb][:]
                ).then_inc(so1, 16)
            sync.wait_ge(so1, 32)

        @block.scalar
        def _(scalar):
            for p in range(4):
                scalar.dma_start(
                    rts[p][:], rp_d[:, xoff[p] * RANK:xoff[p + 1] * RANK]
                ).then_inc(srs[p], 16)
            for cb in range(2, 4):
                scalar.wait_ge(sv, 5 + cb)
                scalar.dma_start(
                    out_d[:, cb * 256:(cb + 1) * 256], osbs[cb][:]
                ).then_inc(so2, 16)
            scalar.wait_ge(so2, 32)

        def stage1(t, pieces, k0):
            """PSUM-accumulate R_p.T.T @ x_p.T for the given pieces."""
            k = k0
            last_mm = None
            for p in pieces:
                t.wait_ge(sxs[p], 16)            # x piece p
                t.wait_ge(srs[p], 16)            # R piece p
                for kl in range(PIECES[p]):
                    last_mm = nc.tensor.matmul(
                        ps1[0:RANK, 0:SSH],
                        rts[p][:, kl * RANK:(kl + 1) * RANK],
                        xts[p][:, kl * SSH:(kl + 1) * SSH],
                        start=(k == k0), stop=(k == k0 + 31),
                    )
                    k += 1
            return last_mm

        def stage3(t, h, sm_base):
            """Quadrant-packed out += t2s[h].T @ C.T; h=0 opens the
            accumulation groups, h=1 closes them and signals vector."""
            for cb in range(4):
                rh, hb = cb // 2, cb % 2
                last_mm = None
                for p in range(2):
                    for w in range(2):
                        q = p * 2 + w
                        c0 = w * 512 + hb * 256
                        last_mm = nc.tensor.matmul(
                            psos[cb][q * SSH:(q + 1) * SSH, 0:256],
                            t2s[h][rh * 64:(rh + 1) * 64, :],
                            cts[p][rh * 64:(rh + 1) * 64, c0:c0 + 256],
                            start=(h == 0), stop=(h == 1),
                            tile_position=(rh * 64, q * SSH),
                        )
                if h == 1:
                    last_mm.then_inc(sm, 1)      # sm=sm_base+cb

        @block.tensor
        def _(t):
            stage1(t, (0, 1), 0).then_inc(sm, 1)            # sm=1: t1a ready
            t.wait_ge(sv, 1)                                # t1a copied
            t.wait_ge(su, 16)                               # uqt loaded
            nc.tensor.matmul(ps2[:, 0:SSH], uqt[:], t1s[0][:],
                             start=True, stop=True).then_inc(sm, 1)  # sm=2
            t.wait_ge(sv, 2)                                # t2a copied
            t.wait_ge(sc, 32)                               # ct0+ct1 loaded
            stage3(t, 0, None)                              # open accum groups
            stage1(t, (2, 3), 32).then_inc(sm, 1)           # sm=3: t1b ready
            t.wait_ge(sv, 3)                                # t1b copied
            nc.tensor.matmul(ps2[:, 0:SSH], uqt[:], t1s[1][:],
                             start=True, stop=True).then_inc(sm, 1)  # sm=4
            t.wait_ge(sv, 4)                                # t2b copied
            stage3(t, 1, 5)                                 # close; sm=5..8

        @block.vector
        def _(v):
            for h in range(2):
                v.wait_ge(sm, 1 + 2 * h)
                nc.vector.tensor_copy(
                    t1s[h][:], ps1[0:RANK, 0:SSH]
                ).then_inc(sv, 1)
                v.wait_ge(sm, 2 + 2 * h)
                nc.vector.tensor_copy(
                    t2s[h][:], ps2[:, 0:SSH]
                ).then_inc(sv, 1)
            for cb in range(4):
                v.wait_ge(sm, 5 + cb)
                nc.vector.tensor_copy(
                    osbs[cb][:], psos[cb][:, 0:256]
                ).then_inc(sv, 1)

    _NC_CACHE["nc"] = nc
    return nc


def _shard_inputs(x, C, U, R):
    xf = np.asarray(x, np.float32).reshape(B * S, M)
    C = np.asarray(C, np.float32)
    U = np.asarray(U, np.float32)
    R = np.asarray(R, np.float32)

    # rp[p, k*64+r] = R[r, 128k+p]
    rp = np.ascontiguousarray(
        R.reshape(RANK, KCH, 128).transpose(2, 1, 0)
    ).reshape(128, KCH * RANK)
    # uq = U.T duplicated along columns: stage 2's lhsT, M=128 so t2.T lands
    # duplicated in both partition halves (stage 3 reads them as row halves)
    uq = np.ascontiguousarray(np.concatenate([U.T, U.T], axis=1))

    in_maps = []
    for c in range(NCORES):
        i, j = divmod(c, NB)
        xs = xf[i * SSH:(i + 1) * SSH, :]
        # xp[p, k*32+s] = xs[s, 128k+p]
        xp = np.ascontiguousarray(
            xs.reshape(SSH, KCH, 128).transpose(2, 1, 0)
        ).reshape(128, KCH * SSH)
        # ct rows 0:64 = C.T cols [0,2048) of this n-shard, rows 64:128 =
        # cols [2048,4096) -- full 128-partition (= full-bandwidth) DMA
        cT = C[j * NSH:(j + 1) * NSH, :].T  # [64, 4096]
        ct = np.ascontiguousarray(
            np.concatenate([cT[:, :2048], cT[:, 2048:]], axis=0)
        )  # [128, 2048]
        in_maps.append({"xp": xp, "rp": rp, "uq": uq, "ct": ct})
    return in_maps


def _unshard_output(core_outs):
    full = np.empty((B * S, N), np.float32)
    for c in range(NCORES):
        i, j = divmod(c, NB)
        q = core_outs[c]  # [128, 1024]: q[32a+s, 512h+nr] = out[s, (4h+a)*512+nr]
        blk = q.reshape(4, SSH, 2, 512).transpose(1, 2, 0, 3).reshape(SSH, NSH)
        full[i * SSH:(i + 1) * SSH, j * NSH:(j + 1) * NSH] = blk
    return full.reshape(B, S, N)


def _ensure_ntff_hook():
    """bass_utils' axon trace path imports antenv.axon_hooks, which this
    container's antenv lacks. Register an equivalent module backed by the
    boot package's ctypes NTFF hook so trace=True (or BASS_TRACE=1) works."""
    import sys
    import types

    try:
        from antenv.axon_hooks import get_axon_ntff_profile_hook  # noqa: F401
        return
    except ImportError:
        pass
    try:
        from trn_agent_boot.trn_boot import _ntff_profile_via_ctypes

        hook = _ntff_profile_via_ctypes("/opt/axon/libaxon_pjrt.so")
    except Exception:
        hook = None
    mod = types.ModuleType("antenv.axon_hooks")
    state = {"hook": hook}
    mod.get_axon_ntff_profile_hook = lambda: state["hook"]
    mod.set_axon_ntff_profile_hook = lambda h: state.update(hook=h)
    sys.modules["antenv.axon_hooks"] = mod


def run(x, C, U, R, trace=False, **spmd_kwargs):
    from concourse.bass_utils import run_bass_kernel_spmd

    _ensure_ntff_hook()
    nc = _build_nc()
    in_maps = _shard_inputs(x, C, U, R)
    res = run_bass_kernel_spmd(
        nc, in_maps, core_ids=list(range(NCORES)), trace=trace, **spmd_kwargs
    )
    out = _unshard_output([r["out"] for r in res.results])
    return out, res


def kernel(x, C, U, R):
    out, _ = run(x, C, U, R, trace=False)
    return out
